# revision 1
# baseline (speedup 1.0000x reference)
"""MultiHeadAttention Trainium2 kernel, 8-core SPMD.

Sharding: core = (batch b, head-group g), b in {0,1}, g in {0..3}.
Each core computes 4 heads of one batch (tensor-parallel on heads,
data-parallel on batch). Out-projection partials and the output bias
are summed on host.

Measured HW design decisions (axon TRN2 microbenchmarks):
- all matmuls bf16: HW streams bf16 ~2.4x faster than float32r
  (87.6 vs 213 ns per [128c,128m,512f] matmul); fp32 PSUM accumulate
- ACT (scalar) engine only runs exp, one [128, 2, 512-rel] instruction
  per head-pair j-tile (head pairing halves ACT instruction count)
- PSUM->SBUF drains on DVE; softmax normalization reciprocal+mul on DVE
- out-projection emitted after each 512-row chunk of attention so its
  PE work and y DMA overlap the ACT-bound attention of later chunks
- phases 1a/1b share pools so the K weight load and second xT stream
  flow during phase-1a compute

Self-contained: hardcodes shapes B=2, S=2048, D=2048, H=16.
"""

import numpy as np
import ml_dtypes

import concourse.bacc as bacc
import concourse.mybir as mybir
import concourse.tile as tile
from concourse.bass_utils import run_bass_kernel_spmd

B, S, D = 2, 2048, 2048
H = 16
HD = D // H          # 128 head dim
G = 4                # head groups (tensor parallel degree)
HPG = H // G         # 4 heads per group
DG = HPG * HD        # 512 features per group
NCORES = 8
NTC = D // 128       # 16 contraction chunks
NIT = S // 128       # 16 seq tiles of 128
NSC = S // 512       # 4 seq chunks of 512
SCALE = float(1.0 / np.sqrt(np.float32(S)))

F32 = mybir.dt.float32
BF16 = mybir.dt.bfloat16
EXP = mybir.ActivationFunctionType.Exp
NPBF16 = ml_dtypes.bfloat16

_CACHE = {}


def _build(nreps=1, trace_sim=False, phases="full"):
    do_2 = phases in ("12", "full")
    do_3 = phases == "full"
    nc = bacc.Bacc(target_bir_lowering=False, trn_type="TRN2")
    xT = nc.dram_tensor("xT", [D, S], BF16, kind="ExternalInput")
    wqT = nc.dram_tensor("wqT", [D, DG], BF16, kind="ExternalInput")
    wkT = nc.dram_tensor("wkT", [D, DG], BF16, kind="ExternalInput")
    wvT = nc.dram_tensor("wvT", [D, DG], BF16, kind="ExternalInput")
    woT = nc.dram_tensor("woT", [DG, D], BF16, kind="ExternalInput")
    mask = nc.dram_tensor("mask", [128, 256], BF16, kind="ExternalInput")
    ones = nc.dram_tensor("ones", [128, 128], BF16, kind="ExternalInput")
    y = nc.dram_tensor("y", [S, D], F32, kind="ExternalOutput")

    with tile.TileContext(nc, trace_sim=trace_sim) as tc:
      for _rep in range(nreps):
        with tc.tile_pool(name="res", bufs=1) as res:
            qt = [res.tile([128, S], BF16, tag=f"qt{h}", name=f"qt{h}") for h in range(HPG)]
            kt = [res.tile([128, S], BF16, tag=f"kt{h}", name=f"kt{h}") for h in range(HPG)]
            vg = [res.tile([128, 4 * DG], BF16, tag=f"vg{j}", name=f"vg{j}") for j in range(4)]
            mask_t = res.tile([128, 256], BF16, tag="mask", name="mask_t")
            ones_t = res.tile([128, 128], BF16, tag="ones", name="ones_t")
            nc.scalar.dma_start(mask_t[:], mask[:])
            nc.scalar.dma_start(ones_t[:], ones[:])

            # ---- Phase 1: Q+V over one xT stream, then K over a second.
            # Shared pools keep the K weight load + second stream flowing
            # during phase-1a compute. All PSUM drains on DVE.
            with tc.tile_pool(name="wts", bufs=1) as wp:
                wqg = [wp.tile([128, 4 * DG], BF16, tag=f"wqg{g}", name=f"wqg{g}") for g in range(4)]
                wvg = [wp.tile([128, 4 * DG], BF16, tag=f"wvg{g}", name=f"wvg{g}") for g in range(4)]
                wkg = [wp.tile([128, 4 * DG], BF16, tag=f"wkg{g}", name=f"wkg{g}") for g in range(4)]
                with (
                    tc.tile_pool(name="xts", bufs=3) as xp,
                    tc.tile_pool(name="ps1", bufs=8, space="PSUM") as pp1,
                ):
                    # -- 1a: Q + V --
                    for ic in range(NSC):
                        i0 = ic * 512
                        qps = [pp1.tile([128, 512], F32, tag="projps", name="projps") for _ in range(HPG)]
                        vps = [pp1.tile([128, DG], F32, tag="projps", name="projps") for _ in range(4)]
                        for g4 in range(4):
                            if ic == 0 and g4 == 0:
                                pass  # first-chunk loads split below
                            elif ic == 0:
                                nc.scalar.dma_start(
                                    wqg[g4][:].rearrange("p (g d) -> p g d", g=4),
                                    wqT[g4 * 512 : (g4 + 1) * 512, :].rearrange(
                                        "(g p) d -> p g d", p=128
                                    ),
                                )
                                nc.scalar.dma_start(
                                    wvg[g4][:].rearrange("p (g d) -> p g d", g=4),
                                    wvT[g4 * 512 : (g4 + 1) * 512, :].rearrange(
                                        "(g p) d -> p g d", p=128
                                    ),
                                )
                            xtg = xp.tile([128, 4 * 512], BF16, tag="xt", name="xt")
                            if ic == 0 and g4 == 0:
                                for g in range(4):
                                    r0 = g * 128
                                    nc.sync.dma_start(
                                        wqg[0][:, g * 512 : (g + 1) * 512],
                                        wqT[r0 : r0 + 128, :],
                                    )
                                    nc.sync.dma_start(
                                        wvg[0][:, g * 512 : (g + 1) * 512],
                                        wvT[r0 : r0 + 128, :],
                                    )
                                    nc.sync.dma_start(
                                        xtg[:, g * 512 : (g + 1) * 512],
                                        xT[r0 : r0 + 128, i0 : i0 + 512],
                                    )
                            else:
                                nc.sync.dma_start(
                                    xtg[:].rearrange("p (g i) -> p g i", g=4),
                                    xT[g4 * 512 : (g4 + 1) * 512, i0 : i0 + 512].rearrange(
                                        "(g p) i -> p g i", p=128
                                    ),
                                )
                            for g in range(4):
                                c = g4 * 4 + g
                                st = c == 0
                                sp = c == NTC - 1
                                xt_c = xtg[:, g * 512 : (g + 1) * 512]
                                wslice = slice(g * 512, (g + 1) * 512)
                                for h in range(HPG):
                                    nc.tensor.matmul(
                                        qps[h][:],
                                        wqg[g4][:, g * 512 + h * 128 : g * 512 + (h + 1) * 128],
                                        xt_c,
                                        start=st,
                                        stop=sp,
                                    )
                                for jj in range(4):
                                    nc.tensor.matmul(
                                        vps[jj][:],
                                        xtg[:, g * 512 + jj * 128 : g * 512 + (jj + 1) * 128],
                                        wvg[g4][:, wslice],
                                        start=st,
                                        stop=sp,
                                    )
                        for h in range(HPG):
                            nc.vector.tensor_copy(qt[h][:, i0 : i0 + 512], qps[h][:])
                        for jj in range(4):
                            nc.vector.tensor_copy(
                                vg[ic][:, jj * DG : (jj + 1) * DG], vps[jj][:]
                            )
                    # -- 1b: K (second xT stream; wk prefetched during 1a) --
                    for g4 in range(4):
                        nc.scalar.dma_start(
                            wkg[g4][:].rearrange("p (g d) -> p g d", g=4),
                            wkT[g4 * 512 : (g4 + 1) * 512, :].rearrange(
                                "(g p) d -> p g d", p=128
                            ),
                        )
                    for ic in range(NSC):
                        i0 = ic * 512
                        kps = [pp1.tile([128, 512], F32, tag="projps", name="projps") for _ in range(HPG)]
                        for g4 in range(4):
                            xtg = xp.tile([128, 4 * 512], BF16, tag="xt", name="xt")
                            nc.sync.dma_start(
                                xtg[:].rearrange("p (g i) -> p g i", g=4),
                                xT[g4 * 512 : (g4 + 1) * 512, i0 : i0 + 512].rearrange(
                                    "(g p) i -> p g i", p=128
                                ),
                            )
                            for g in range(4):
                                c = g4 * 4 + g
                                st = c == 0
                                sp = c == NTC - 1
                                xt_c = xtg[:, g * 512 : (g + 1) * 512]
                                for h in range(HPG):
                                    nc.tensor.matmul(
                                        kps[h][:],
                                        wkg[g4][:, g * 512 + h * 128 : g * 512 + (h + 1) * 128],
                                        xt_c,
                                        start=st,
                                        stop=sp,
                                    )
                        for h in range(HPG):
                            nc.vector.tensor_copy(kt[h][:, i0 : i0 + 512], kps[h][:])

            # ---- Phase 2+3 merged: attention and out-proj per 512-chunk.
            if not do_2:
                continue
            with tc.tile_pool(name="p2res", bufs=1) as p2r:
                ctxt = [p2r.tile([128, S], BF16, tag=f"ctx{h}", name=f"ctx{h}") for h in range(HPG)]
                wo = [p2r.tile([128, D], BF16, tag=f"wo{h}", name=f"wo{h}") for h in range(HPG)]
                for h in range(HPG):
                    nc.sync.dma_start(wo[h][:], woT[h * 128 : (h + 1) * 128, :])

                with (
                    tc.tile_pool(name="etp", bufs=4) as etp,
                    tc.tile_pool(name="ysbp", bufs=2) as yp,
                    tc.tile_pool(name="ps2", bufs=1, space="PSUM") as psp,
                ):
                    for ic in range(NSC):
                        i0 = ic * 512
                        nj = 4 * (ic + 1)
                        for hp in range(2):  # head pairs (0,1), (2,3)
                            h0, h1 = 2 * hp, 2 * hp + 1
                            cps = [
                                psp.tile([128, 512], F32, tag=f"ctxps{t}", name="cps", bufs=1)
                                for t in range(2)
                            ]
                            rps = [
                                psp.tile([128, 512], F32, tag=f"rsps{t}", name="rps", bufs=1)
                                for t in range(2)
                            ]
                            for jb in range(nj):
                                j0 = jb * 128
                                ist = max(i0, j0)
                                rel = ist - i0
                                stp2 = psp.tile(
                                    [128, 1024], F32, tag="stp2", name="stp2", bufs=2
                                )
                                for t, h in enumerate((h0, h1)):
                                    nc.tensor.matmul(
                                        stp2[:, t * 512 + rel : (t + 1) * 512],
                                        kt[h][:, j0 : j0 + 128],
                                        qt[h][:, ist : i0 + 512],
                                        start=True, stop=True,
                                    )
                                et2 = etp.tile([128, 1024], BF16, tag="et2", name="et2")
                                nc.scalar.activation(
                                    et2[:].rearrange("p (t i) -> p t i", t=2)[:, :, rel:512],
                                    stp2[:].rearrange("p (t i) -> p t i", t=2)[:, :, rel:512],
                                    EXP, bias=0.0, scale=SCALE,
                                )
                                if j0 >= i0:
                                    nc.gpsimd.tensor_mul(
                                        et2[:].rearrange("p (t i) -> p t i", t=2)[
                                            :, :, rel : rel + 128
                                        ],
                                        et2[:].rearrange("p (t i) -> p t i", t=2)[
                                            :, :, rel : rel + 128
                                        ],
                                        mask_t[:].rearrange("p (t j) -> p t j", t=2),
                                    )
                                for t, h in enumerate((h0, h1)):
                                    nc.tensor.matmul(
                                        cps[t][:, rel:512],
                                        vg[jb // 4][
                                            :,
                                            (jb % 4) * DG + h * 128 : (jb % 4) * DG
                                            + (h + 1) * 128,
                                        ],
                                        et2[:, t * 512 + rel : (t + 1) * 512],
                                        start=(jb == 0), stop=(jb == nj - 1),
                                    )
                                for t in range(2):
                                    nc.tensor.matmul(
                                        rps[t][:, rel:512],
                                        ones_t[:],
                                        et2[:, t * 512 + rel : (t + 1) * 512],
                                        start=(jb == 0), stop=(jb == nj - 1),
                                    )
                            for t, h in enumerate((h0, h1)):
                                rrb = etp.tile([128, 512], F32, tag="rrb", name="rrb", bufs=2)
                                nc.vector.reciprocal_approx_fast(rrb[:], rps[t][:])
                                nc.vector.tensor_mul(
                                    ctxt[h][:, i0 : i0 + 512], cps[t][:], rrb[:]
                                )
                        # -- out-proj for this chunk's 4 i-tiles --
                        if do_3:
                            for itl in range(4):
                                t0 = i0 + itl * 128
                                ysb = yp.tile([128, D], F32, tag="ysb", name="ysb")
                                for op in range(2):  # 1024-wide halves
                                    yps2 = psp.tile(
                                        [128, 1024], F32, tag="stp2", name="yps2", bufs=2
                                    )
                                    for oc in range(2):
                                        o0 = op * 1024 + oc * 512
                                        for h in range(HPG):
                                            nc.tensor.matmul(
                                                yps2[:, oc * 512 : (oc + 1) * 512],
                                                ctxt[h][:, t0 : t0 + 128],
                                                wo[h][:, o0 : o0 + 512],
                                                start=(h == 0), stop=(h == HPG - 1),
                                            )
                                    nc.vector.tensor_copy(
                                        ysb[:, op * 1024 : (op + 1) * 1024], yps2[:]
                                    )
                                nc.sync.dma_start(y[t0 : t0 + 128, :], ysb[:])
    nc.finalize()
    return nc


def get_nc():
    if "nc" not in _CACHE:
        _CACHE["nc"] = _build()
    return _CACHE["nc"]


def make_in_maps(inputs, w_q, w_k, w_v, w_o, b_o):
    x = np.asarray(inputs, dtype=np.float32)
    w_q = np.asarray(w_q, dtype=np.float32)
    w_k = np.asarray(w_k, dtype=np.float32)
    w_v = np.asarray(w_v, dtype=np.float32)
    w_o = np.asarray(w_o, dtype=np.float32)

    mask = np.tile(np.triu(np.ones((128, 128), dtype=np.float32)), (1, 2)).astype(NPBF16)
    ones = np.ones((128, 128), dtype=NPBF16)

    xTs = [np.ascontiguousarray(x[b].T).astype(NPBF16) for b in range(B)]
    wqTs = [np.ascontiguousarray(w_q[g * DG : (g + 1) * DG, :].T).astype(NPBF16) for g in range(G)]
    wkTs = [np.ascontiguousarray(w_k[g * DG : (g + 1) * DG, :].T).astype(NPBF16) for g in range(G)]
    wvTs = [np.ascontiguousarray(w_v[g * DG : (g + 1) * DG, :].T).astype(NPBF16) for g in range(G)]
    woTs = [np.ascontiguousarray(w_o[:, g * DG : (g + 1) * DG].T).astype(NPBF16) for g in range(G)]

    in_maps = []
    for core in range(NCORES):
        b, g = divmod(core, G)
        in_maps.append(
            {
                "xT": xTs[b],
                "wqT": wqTs[g],
                "wkT": wkTs[g],
                "wvT": wvTs[g],
                "woT": woTs[g],
                "mask": mask,
                "ones": ones,
            }
        )
    return in_maps


def assemble(results, b_o):
    out = np.zeros((B, S, D), dtype=np.float32)
    for core in range(NCORES):
        b = core // G
        out[b] += results[core]["y"].astype(np.float32)
    out += np.asarray(b_o, dtype=np.float32)[None, None, :]
    return out


def kernel(inputs, w_q, w_k, w_v, w_o, b_o):
    nc = get_nc()
    in_maps = make_in_maps(inputs, w_q, w_k, w_v, w_o, b_o)
    res = run_bass_kernel_spmd(nc, in_maps, core_ids=list(range(NCORES)))
    return assemble(res.results, b_o)



# revision 4
# speedup vs baseline: 1.2214x; 1.2214x over previous
"""MultiHeadAttention Trainium2 kernel, 8-core SPMD.

Sharding: core = (batch b, head-group g), b in {0,1}, g in {0..3}.
Each core computes 4 heads of one batch (tensor-parallel on heads,
data-parallel on batch). Out-projection partials and the output bias
are summed on host.

Structure (v2, PE-roofline oriented — sim showed PE 88.8% busy is the
bottleneck and the ones-matmul rowsum burned ~10% of PE):
- single merged QKV pass per 512-token chunk over ONE xT stream
  (xT tiles held in SBUF for the chunk; K no longer needs a 2nd pass
  over HBM)
- projection matmuls run in 2-head sub-passes needing only 2 PSUM
  banks, so they interleave as PE "filler" inside the ACT-bound
  attention loop (PSUM: 2 proj/outproj + 4 score + 2 ctx = 8 banks)
- softmax denominators: DVE accumulates exp tiles (A += et) per
  j-block; ONE ones-matmul per (head-pair, chunk) on the accumulated
  [128,1024] tile replaces per-j-block rowsum matmuls (saves ~29us PE)
- out-projection emitted as 4-matmul groups into the same filler
  stream; attention j-loop drains filler units to keep PE saturated
  while ACT runs exp

Self-contained: hardcodes shapes B=2, S=2048, D=2048, H=16.
"""

from collections import deque

import numpy as np
import ml_dtypes

import concourse.bacc as bacc
import concourse.mybir as mybir
import concourse.tile as tile
from concourse.bass_utils import run_bass_kernel_spmd

B, S, D = 2, 2048, 2048
H = 16
HD = D // H          # 128 head dim
G = 4                # head groups (tensor parallel degree)
HPG = H // G         # 4 heads per group
DG = HPG * HD        # 512 features per group
NCORES = 8
NTC = D // 128       # 16 contraction chunks
NSC = S // 512       # 4 seq chunks of 512
SCALE = float(1.0 / np.sqrt(np.float32(S)))

F32 = mybir.dt.float32
BF16 = mybir.dt.bfloat16
EXP = mybir.ActivationFunctionType.Exp
NPBF16 = ml_dtypes.bfloat16

_CACHE = {}


def _build(nreps=1, trace_sim=False):
    nc = bacc.Bacc(target_bir_lowering=False, trn_type="TRN2")
    xT = nc.dram_tensor("xT", [D, S], BF16, kind="ExternalInput")
    wqT = nc.dram_tensor("wqT", [D, DG], BF16, kind="ExternalInput")
    wkT = nc.dram_tensor("wkT", [D, DG], BF16, kind="ExternalInput")
    wvT = nc.dram_tensor("wvT", [D, DG], BF16, kind="ExternalInput")
    woT = nc.dram_tensor("woT", [DG, D], BF16, kind="ExternalInput")
    mask = nc.dram_tensor("mask", [128, 256], BF16, kind="ExternalInput")
    ones = nc.dram_tensor("ones", [128, 128], BF16, kind="ExternalInput")
    y = nc.dram_tensor("y", [S, D], F32, kind="ExternalOutput")

    with tile.TileContext(nc, trace_sim=trace_sim) as tc:
      for _rep in range(nreps):
        with (
            tc.tile_pool(name="res", bufs=1) as res,
            tc.tile_pool(name="xts", bufs=1) as xp,
            tc.tile_pool(name="wrk", bufs=1) as wk,
            tc.tile_pool(name="ps", bufs=1, space="PSUM") as pp,
        ):
            qt = [res.tile([128, S], BF16, tag=f"qt{h}", name=f"qt{h}") for h in range(HPG)]
            kt = [res.tile([128, S], BF16, tag=f"kt{h}", name=f"kt{h}") for h in range(HPG)]
            vg = [res.tile([128, 4 * DG], BF16, tag=f"vg{j}", name=f"vg{j}") for j in range(4)]
            ctxt = [res.tile([128, S], BF16, tag=f"ctx{h}", name=f"ctx{h}") for h in range(HPG)]
            wo = [res.tile([128, D], BF16, tag=f"wo{h}", name=f"wo{h}") for h in range(HPG)]
            wqg = [res.tile([128, 4 * DG], BF16, tag=f"wqg{g}", name=f"wqg{g}") for g in range(4)]
            wvg = [res.tile([128, 4 * DG], BF16, tag=f"wvg{g}", name=f"wvg{g}") for g in range(4)]
            wkg = [res.tile([128, 4 * DG], BF16, tag=f"wkg{g}", name=f"wkg{g}") for g in range(4)]
            mask_t = res.tile([128, 256], BF16, tag="mask", name="mask_t")
            ones_t = res.tile([128, 128], BF16, tag="ones", name="ones_t")

            # ---- upfront DMAs (scalar queue for weights/consts) ----
            # first-needed pieces first: wq g4=0 split per 128-row block
            for g in range(4):
                nc.scalar.dma_start(
                    wqg[0][:, g * 512 : (g + 1) * 512], wqT[g * 128 : (g + 1) * 128, :]
                )
            nc.scalar.dma_start(mask_t[:], mask[:])
            nc.scalar.dma_start(ones_t[:], ones[:])
            for g4 in range(1, 4):
                nc.scalar.dma_start(
                    wqg[g4][:].rearrange("p (g d) -> p g d", g=4),
                    wqT[g4 * 512 : (g4 + 1) * 512, :].rearrange("(g p) d -> p g d", p=128),
                )
            for g4 in range(4):
                nc.scalar.dma_start(
                    wvg[g4][:].rearrange("p (g d) -> p g d", g=4),
                    wvT[g4 * 512 : (g4 + 1) * 512, :].rearrange("(g p) d -> p g d", p=128),
                )
            for g4 in range(4):
                nc.scalar.dma_start(
                    wkg[g4][:].rearrange("p (g d) -> p g d", g=4),
                    wkT[g4 * 512 : (g4 + 1) * 512, :].rearrange("(g p) d -> p g d", p=128),
                )
            for h in range(HPG):
                nc.scalar.dma_start(wo[h][:], woT[h * 128 : (h + 1) * 128, :])

            chx = {}  # chunk -> list of 4 xtg tiles

            def load_x(ic, split_first=False):
                i0 = ic * 512
                tiles = []
                for g4 in range(4):
                    t = xp.tile([128, 4 * 512], BF16, tag=f"xt{g4}", name=f"xt{g4}", bufs=2)
                    if split_first and g4 == 0:
                        for g in range(4):
                            r0 = g * 128
                            nc.sync.dma_start(
                                t[:, g * 512 : (g + 1) * 512],
                                xT[r0 : r0 + 128, i0 : i0 + 512],
                            )
                    else:
                        nc.sync.dma_start(
                            t[:].rearrange("p (g i) -> p g i", g=4),
                            xT[g4 * 512 : (g4 + 1) * 512, i0 : i0 + 512].rearrange(
                                "(g p) i -> p g i", p=128
                            ),
                        )
                    tiles.append(t)
                chx[ic] = tiles

            # ---- filler units: closures emitting ~4 PE matmuls each ----
            filler = deque()  # items: (kind, fn); kind "jX" must flush before A(X)

            def push_qvk(ic):
                i0 = ic * 512
                for kind, sub in (
                    ("q", 0), ("q", 1), ("v", 0), ("v", 1),
                    ("k", 0), ("k", 1),
                ):
                    st = {}
                    for u0 in range(8):
                        def emit(kind=kind, sub=sub, u0=u0, st=st):
                            if u0 == 0:
                                st["ps"] = [
                                    pp.tile([128, 512], F32, tag="P", name="P", bufs=2)
                                    for _ in range(2)
                                ]
                            ps = st["ps"]
                            xts = chx[ic]
                            for s in range(2 * u0, 2 * u0 + 2):
                                g4, g = divmod(s, 4)
                                stt = s == 0
                                sp = s == NTC - 1
                                xt_c = xts[g4][:, g * 512 : (g + 1) * 512]
                                for t in range(2):
                                    idx = 2 * sub + t
                                    if kind == "q":
                                        nc.tensor.matmul(
                                            ps[t][:],
                                            wqg[g4][:, g * 512 + idx * 128 : g * 512 + (idx + 1) * 128],
                                            xt_c, start=stt, stop=sp,
                                        )
                                    elif kind == "k":
                                        nc.tensor.matmul(
                                            ps[t][:],
                                            wkg[g4][:, g * 512 + idx * 128 : g * 512 + (idx + 1) * 128],
                                            xt_c, start=stt, stop=sp,
                                        )
                                    else:
                                        nc.tensor.matmul(
                                            ps[t][:],
                                            xts[g4][:, g * 512 + idx * 128 : g * 512 + (idx + 1) * 128],
                                            wvg[g4][:, g * 512 : (g + 1) * 512],
                                            start=stt, stop=sp,
                                        )
                            if u0 == 7:
                                for t in range(2):
                                    idx = 2 * sub + t
                                    if kind == "q":
                                        nc.vector.tensor_copy(qt[idx][:, i0 : i0 + 512], ps[t][:])
                                    elif kind == "k":
                                        nc.vector.tensor_copy(kt[idx][:, i0 : i0 + 512], ps[t][:])
                                    else:
                                        nc.vector.tensor_copy(
                                            vg[ic][:, idx * DG : (idx + 1) * DG], ps[t][:]
                                        )
                        filler.append((f"j{ic}", emit))

            def push_oproj(ic):
                i0 = ic * 512
                for itl in range(4):
                    st = {}
                    for oc in range(4):
                        def emit(itl=itl, oc=oc, st=st):
                            t0 = i0 + itl * 128
                            if oc == 0:
                                st["ysb"] = wk.tile([128, D], F32, tag="ysb", name="ysb", bufs=2)
                            yps = pp.tile([128, 512], F32, tag="P", name="P", bufs=2)
                            o0 = oc * 512
                            for h in range(HPG):
                                nc.tensor.matmul(
                                    yps[:],
                                    ctxt[h][:, t0 : t0 + 128],
                                    wo[h][:, o0 : o0 + 512],
                                    start=(h == 0), stop=(h == HPG - 1),
                                )
                            nc.vector.tensor_copy(st["ysb"][:, o0 : o0 + 512], yps[:])
                            if oc == 3:
                                nc.sync.dma_start(y[t0 : t0 + 128, :], st["ysb"][:])
                        filler.append(("o", emit))

            def drain(n):
                for _ in range(n):
                    if not filler:
                        return
                    filler.popleft()[1]()

            def flush_chunk(ic):
                # emit everything remaining that A(ic) depends on
                while filler and any(k == f"j{ic}" for k, _ in filler):
                    filler.popleft()[1]()

            # ---- emission ----
            load_x(0, split_first=True)
            push_qvk(0)
            flush_chunk(0)

            for ic in range(NSC):
                i0 = ic * 512
                nj = 4 * (ic + 1)
                if ic + 1 < NSC:
                    load_x(ic + 1)
                    push_qvk(ic + 1)
                # ---- attention for chunk ic ----
                for hp in range(2):
                    h0, h1 = 2 * hp, 2 * hp + 1
                    cps = [
                        pp.tile([128, 512], F32, tag=f"C{t}", name="cps", bufs=1)
                        for t in range(2)
                    ]
                    acc = wk.tile([128, 1024], BF16, tag="A", name="acc", bufs=2)
                    for jb in range(nj):
                        j0 = jb * 128
                        ist = max(i0, j0)
                        rel = ist - i0
                        stp = pp.tile([128, 1024], F32, tag="S", name="stp", bufs=2)
                        for t, h in enumerate((h0, h1)):
                            nc.tensor.matmul(
                                stp[:, t * 512 + rel : (t + 1) * 512],
                                kt[h][:, j0 : j0 + 128],
                                qt[h][:, ist : i0 + 512],
                                start=True, stop=True,
                            )
                        et = wk.tile([128, 1024], BF16, tag="et", name="et", bufs=4)
                        nc.scalar.activation(
                            et[:].rearrange("p (t i) -> p t i", t=2)[:, :, rel:512],
                            stp[:].rearrange("p (t i) -> p t i", t=2)[:, :, rel:512],
                            EXP, bias=0.0, scale=SCALE,
                        )
                        if j0 >= i0:
                            nc.gpsimd.tensor_mul(
                                et[:].rearrange("p (t i) -> p t i", t=2)[:, :, rel : rel + 128],
                                et[:].rearrange("p (t i) -> p t i", t=2)[:, :, rel : rel + 128],
                                mask_t[:].rearrange("p (t j) -> p t j", t=2),
                            )
                        for t, h in enumerate((h0, h1)):
                            nc.tensor.matmul(
                                cps[t][:, rel:512],
                                vg[jb // 4][
                                    :, (jb % 4) * DG + h * 128 : (jb % 4) * DG + (h + 1) * 128
                                ],
                                et[:, t * 512 + rel : (t + 1) * 512],
                                start=(jb == 0), stop=(jb == nj - 1),
                            )
                        if jb == 0:
                            nc.vector.tensor_copy(acc[:], et[:])
                        else:
                            nc.vector.tensor_add(
                                acc[:].rearrange("p (t i) -> p t i", t=2)[:, :, rel:512],
                                acc[:].rearrange("p (t i) -> p t i", t=2)[:, :, rel:512],
                                et[:].rearrange("p (t i) -> p t i", t=2)[:, :, rel:512],
                            )
                        drain(2 if ic + 1 < NSC else jb % 2)
                    # denominators: one partition-sum matmul on accumulated tile
                    dps = pp.tile([128, 1024], F32, tag="S", name="dps", bufs=2)
                    for t in range(2):
                        nc.tensor.matmul(
                            dps[:, t * 512 : (t + 1) * 512],
                            ones_t[:],
                            acc[:, t * 512 : (t + 1) * 512],
                            start=True, stop=True,
                        )
                    rrb = wk.tile([128, 1024], F32, tag="R", name="rrb", bufs=2)
                    nc.vector.reciprocal_approx_fast(rrb[:], dps[:])
                    for t, h in enumerate((h0, h1)):
                        nc.vector.tensor_mul(
                            ctxt[h][:, i0 : i0 + 512], cps[t][:],
                            rrb[:, t * 512 : (t + 1) * 512],
                        )
                push_oproj(ic)
                if ic + 1 < NSC:
                    flush_chunk(ic + 1)
            drain(len(filler) + 1)
    nc.finalize()
    return nc


def get_nc():
    if "nc" not in _CACHE:
        _CACHE["nc"] = _build()
    return _CACHE["nc"]


def make_in_maps(inputs, w_q, w_k, w_v, w_o, b_o):
    x = np.asarray(inputs, dtype=np.float32)
    w_q = np.asarray(w_q, dtype=np.float32)
    w_k = np.asarray(w_k, dtype=np.float32)
    w_v = np.asarray(w_v, dtype=np.float32)
    w_o = np.asarray(w_o, dtype=np.float32)

    mask = np.tile(np.triu(np.ones((128, 128), dtype=np.float32)), (1, 2)).astype(NPBF16)
    ones = np.ones((128, 128), dtype=NPBF16)

    xTs = [np.ascontiguousarray(x[b].T).astype(NPBF16) for b in range(B)]
    wqTs = [np.ascontiguousarray(w_q[g * DG : (g + 1) * DG, :].T).astype(NPBF16) for g in range(G)]
    wkTs = [np.ascontiguousarray(w_k[g * DG : (g + 1) * DG, :].T).astype(NPBF16) for g in range(G)]
    wvTs = [np.ascontiguousarray(w_v[g * DG : (g + 1) * DG, :].T).astype(NPBF16) for g in range(G)]
    woTs = [np.ascontiguousarray(w_o[:, g * DG : (g + 1) * DG].T).astype(NPBF16) for g in range(G)]

    in_maps = []
    for core in range(NCORES):
        b, g = divmod(core, G)
        in_maps.append(
            {
                "xT": xTs[b],
                "wqT": wqTs[g],
                "wkT": wkTs[g],
                "wvT": wvTs[g],
                "woT": woTs[g],
                "mask": mask,
                "ones": ones,
            }
        )
    return in_maps


def assemble(results, b_o):
    out = np.zeros((B, S, D), dtype=np.float32)
    for core in range(NCORES):
        b = core // G
        out[b] += results[core]["y"].astype(np.float32)
    out += np.asarray(b_o, dtype=np.float32)[None, None, :]
    return out


def kernel(inputs, w_q, w_k, w_v, w_o, b_o):
    nc = get_nc()
    in_maps = make_in_maps(inputs, w_q, w_k, w_v, w_o, b_o)
    res = run_bass_kernel_spmd(nc, in_maps, core_ids=list(range(NCORES)))
    return assemble(res.results, b_o)
